# revision 2
# baseline (speedup 1.0000x reference)
"""Trainium2 Bass kernel: unscaled full attention.

    out = softmax(（x@wq) @ (x@wk).T) @ (x@wv)      x:[N,D] f32, w*:[D,D] f32

Distribution (8 NeuronCores, sequence parallel):
  - queries sharded over cores (NL = N/8 rows each), weights replicated
  - each core computes its K/V shard, one AllGather shares K^T and V
  - per-core flash-style attention with deferred softmax rescaling

Precision: fp16 inputs / Q / K / P / V with f32 PSUM accumulation and f32
softmax statistics (measured end-to-end rel err ~4e-3 vs f32 reference;
bf16 was 8x worse because the unscaled logits have std ~45).
"""

import numpy as np

P = 128      # SBUF partitions
JCHW = 512   # score j-chunk width (PSUM bank = 512 f32)
DBW = 512    # output d-block width


class Cfg:
    def __init__(self, N=8192, D=2048, NC=8, GT=2):
        self.N, self.D, self.NC, self.GT = N, D, NC, GT
        self.NL = N // NC            # local (per-core) query rows
        self.DK = D // P             # contraction tiles
        self.NIT = self.NL // P      # i-tiles per core
        self.NG = self.NIT // GT     # groups of GT i-tiles
        self.JCH = min(JCHW, self.NL)  # j-chunk width (never crosses a rank)
        self.NJC = N // self.JCH     # j-chunks
        self.NJS = N // P            # j-subtiles
        self.ND = D // DBW           # output d-blocks
        self.PIC = min(512, self.NL)  # projection i-chunk width
        self.NPIC = self.NL // self.PIC
        assert self.NL % P == 0 and D % P == 0 and self.NIT % GT == 0
        assert self.JCH % P == 0 and self.NL % self.JCH == 0 and D % DBW == 0


def build(cfg):
    import concourse.bass as bass
    import concourse.tile as tile
    from concourse import bacc, mybir
    from concourse.masks import make_identity

    FP16 = mybir.dt.float16
    F32 = mybir.dt.float32
    AX = mybir.AxisListType.X
    ALU = mybir.AluOpType
    EXP = mybir.ActivationFunctionType.Exp

    N, D, NC, GT = cfg.N, cfg.D, cfg.NC, cfg.GT
    NL, DK, NIT, NG = cfg.NL, cfg.DK, cfg.NIT, cfg.NG
    JCH, NJC, NJS, ND = cfg.JCH, cfg.NJC, cfg.NJS, cfg.ND

    nc = bacc.Bacc("TRN2", target_bir_lowering=False, debug=False, num_devices=NC)

    xT = nc.dram_tensor("xT", [D, NL], FP16, kind="ExternalInput").ap()
    wk = nc.dram_tensor("wk", [D, D], FP16, kind="ExternalInput").ap()
    wv = nc.dram_tensor("wv", [D, D], FP16, kind="ExternalInput").ap()
    wq = nc.dram_tensor("wq", [D, D], FP16, kind="ExternalInput").ap()
    out = nc.dram_tensor("out", [NL, D], F32, kind="ExternalOutput").ap()

    with tile.TileContext(nc) as tc:
        with (
            tc.tile_pool(name="persist", bufs=1) as persist,
            tc.tile_pool(name="stats", bufs=1) as statp,
            tc.tile_pool(name="dram", bufs=1, space="DRAM") as dram,
        ):
            qt = persist.tile([P, DK, NL], FP16)        # Q^T resident
            ident = persist.tile([P, P], FP16)
            make_identity(nc, ident)

            kv_in = dram.tile([2, D * NL], FP16)
            kvg = dram.tile([NC, 2, D * NL], FP16, addr_space="Shared")
            ktv = kv_in[0].rearrange("(r c) -> r c", r=D)    # K^T shard [D, NL]
            vv = kv_in[1].rearrange("(r c) -> r c", r=NL)    # V shard  [NL, D]

            # ---------------- projections ----------------
            with (
                tc.tile_pool(name="xtp", bufs=1) as xtp,
                tc.tile_pool(name="wp", bufs=1) as wp,
                tc.tile_pool(name="pstage", bufs=4) as pst,
                tc.tile_pool(name="ppsum", bufs=4, space="PSUM") as pps,
            ):
                xt = xtp.tile([P, DK, NL], FP16)
                nc.sync.dma_start(xt[:], xT.rearrange("(k p) i -> p k i", p=P))

                def proj_T(w_src, sink):
                    # sink(do, ic, psum): consume [P, PIC] f32 tile of W.T@X.T
                    w_t = wp.tile([P, DK, D], FP16, tag="w", name="w_t")
                    nc.sync.dma_start(
                        w_t[:], w_src.rearrange("(k p) o -> p k o", p=P)
                    )
                    for do in range(DK):
                        for ic in range(cfg.NPIC):
                            ps = pps.tile([P, cfg.PIC], F32, tag="pp", name="ps")
                            for dk in range(DK):
                                nc.tensor.matmul(
                                    ps[:],
                                    lhsT=w_t[:, dk, do * P:(do + 1) * P],
                                    rhs=xt[:, dk, ic * cfg.PIC:(ic + 1) * cfg.PIC],
                                    start=(dk == 0),
                                    stop=(dk == DK - 1),
                                )
                            sink(do, ic, ps)
                    return w_t

                def k_sink(do, ic, ps):
                    st = pst.tile([P, cfg.PIC], FP16, tag="pst", name="st")
                    nc.vector.tensor_copy(st[:], ps[:])
                    nc.sync.dma_start(
                        ktv[do * P:(do + 1) * P, ic * cfg.PIC:(ic + 1) * cfg.PIC],
                        st[:],
                    )

                proj_T(wk, k_sink)

                # V in natural [NL, D] layout: lhsT = x^T tile, rhs = w
                w_t = wp.tile([P, DK, D], FP16, tag="w", name="w_t")
                nc.sync.dma_start(w_t[:], wv.rearrange("(k p) o -> p k o", p=P))
                for it in range(NIT):
                    for dc in range(D // DBW):
                        ps = pps.tile([P, DBW], F32, tag="pp", name="ps")
                        for dk in range(DK):
                            nc.tensor.matmul(
                                ps[:],
                                lhsT=xt[:, dk, it * P:(it + 1) * P],
                                rhs=w_t[:, dk, dc * DBW:(dc + 1) * DBW],
                                start=(dk == 0),
                                stop=(dk == DK - 1),
                            )
                        st = pst.tile([P, DBW], FP16, tag="pst", name="st")
                        nc.vector.tensor_copy(st[:], ps[:])
                        nc.sync.dma_start(
                            vv[it * P:(it + 1) * P, dc * DBW:(dc + 1) * DBW], st[:]
                        )

                nc.gpsimd.collective_compute(
                    "AllGather",
                    mybir.AluOpType.bypass,
                    replica_groups=[list(range(NC))],
                    ins=[kv_in.opt()],
                    outs=[kvg.opt()],
                )

                def q_sink(do, ic, ps):
                    nc.scalar.copy(
                        qt[:, do, ic * cfg.PIC:(ic + 1) * cfg.PIC], ps[:]
                    )

                proj_T(wq, q_sink)

            # ---------------- attention ----------------
            mneg = statp.tile([P, NIT, NJC], F32)   # -(chunk max)
            lsum = statp.tile([P, NIT, NJC], F32)   # chunk sum of exp(S - m_c)
            alpha = statp.tile([P, NIT, NJC], F32)  # exp(m_c - M)
            mmin = statp.tile([P, NIT], F32)        # -M (min over chunks of mneg)
            ltot = statp.tile([P, NIT], F32)
            rinv = statp.tile([P, NIT], F32)
            scr = statp.tile([P, NJC], F32)

            with (
                tc.tile_pool(name="pbp", bufs=1) as pbp,
                tc.tile_pool(name="ptp", bufs=1) as ptp,
                tc.tile_pool(name="ktp", bufs=3) as ktp,
                tc.tile_pool(name="vtp", bufs=4) as vtp,
                tc.tile_pool(name="rscp", bufs=3) as rscp,
                tc.tile_pool(name="ostp", bufs=4) as ostp,
                tc.tile_pool(name="spp", bufs=3, space="PSUM") as spp,
                tc.tile_pool(name="tpp", bufs=2, space="PSUM") as tpp,
                tc.tile_pool(name="opp", bufs=3, space="PSUM") as opp,
            ):
                for g in range(NG):
                    # ---- scores + chunk-local softmax ----
                    pb = pbp.tile([P, GT, N], FP16, tag="pb", name="pb")
                    for c in range(NJC):
                        r, off = divmod(c * JCH, NL)
                        ktc = ktp.tile([P, DK, JCH], FP16, tag="kt", name="ktc")
                        nc.sync.dma_start(
                            ktc[:],
                            kvg[r, 0].rearrange(
                                "(k p j) -> p k j", p=P, j=NL
                            )[:, :, off:off + JCH],
                        )
                        for t in range(GT):
                            it = g * GT + t
                            ps = spp.tile([P, JCH], F32, tag="sp", name="ps")
                            for dk in range(DK):
                                nc.tensor.matmul(
                                    ps[:],
                                    lhsT=qt[:, dk, it * P:(it + 1) * P],
                                    rhs=ktc[:, dk, :],
                                    start=(dk == 0),
                                    stop=(dk == DK - 1),
                                )
                            nc.vector.tensor_reduce(
                                out=mneg[:, it, c:c + 1], in_=ps[:],
                                axis=AX, op=ALU.max, negate=True,
                            )
                            nc.scalar.activation(
                                pb[:, t, c * JCH:(c + 1) * JCH], ps[:], EXP,
                                bias=mneg[:, it, c:c + 1], scale=1.0,
                                accum_out=lsum[:, it, c:c + 1],
                            )

                    # ---- global stats: M, alpha, 1/l ----
                    for t in range(GT):
                        it = g * GT + t
                        nc.vector.tensor_reduce(
                            out=mmin[:, it:it + 1], in_=mneg[:, it, :],
                            axis=AX, op=ALU.min,
                        )
                        nc.scalar.activation(
                            alpha[:, it, :], mneg[:, it, :], EXP,
                            bias=mmin[:, it:it + 1], scale=-1.0,
                        )
                        # (tensor_tensor_reduce faults on this runtime; use 2 ops)
                        nc.vector.tensor_mul(scr[:], alpha[:, it, :], lsum[:, it, :])
                        nc.vector.tensor_reduce(
                            out=ltot[:, it:it + 1], in_=scr[:], axis=AX, op=ALU.add
                        )
                        nc.vector.reciprocal(rinv[:, it:it + 1], ltot[:, it:it + 1])

                    # ---- rescale + transpose P ----
                    pt = ptp.tile([P, NJS, GT * P], FP16, tag="pt", name="pt")
                    for t in range(GT):
                        it = g * GT + t
                        for c in range(NJC):
                            rs = rscp.tile([P, JCH], FP16, tag="rs", name="rs")
                            nc.scalar.mul(
                                rs[:], pb[:, t, c * JCH:(c + 1) * JCH],
                                alpha[:, it, c:c + 1],
                            )
                            for jj in range(JCH // P):
                                js = c * (JCH // P) + jj
                                tp = tpp.tile([P, P], FP16, tag="tp", name="tp")
                                nc.tensor.transpose(
                                    tp[:], rs[:, jj * P:(jj + 1) * P], ident[:]
                                )
                                nc.vector.tensor_copy(
                                    pt[:, js, t * P:(t + 1) * P], tp[:]
                                )

                    # ---- P^T @ V, d-block sweeps ----
                    ost = [
                        ostp.tile([P, D], F32, tag="os", name=f"ost{t}")
                        for t in range(GT)
                    ]
                    for db in range(ND):
                        ops = [
                            opp.tile([P, DBW], F32, tag="op", name=f"op{t}")
                            for t in range(GT)
                        ]
                        for js in range(NJS):
                            r, jj = divmod(js, NL // P)
                            vt = vtp.tile([P, DBW], FP16, tag="vt", name="vt")
                            nc.sync.dma_start(
                                vt[:],
                                kvg[r, 1].rearrange("(j d) -> j d", d=D)[
                                    jj * P:(jj + 1) * P, db * DBW:(db + 1) * DBW
                                ],
                            )
                            for t in range(GT):
                                nc.tensor.matmul(
                                    ops[t][:],
                                    lhsT=pt[:, js, t * P:(t + 1) * P],
                                    rhs=vt[:],
                                    start=(js == 0),
                                    stop=(js == NJS - 1),
                                )
                        for t in range(GT):
                            it = g * GT + t
                            nc.vector.tensor_scalar_mul(
                                ost[t][:, db * DBW:(db + 1) * DBW], ops[t][:],
                                rinv[:, it:it + 1],
                            )
                    for t in range(GT):
                        it = g * GT + t
                        nc.sync.dma_start(out[it * P:(it + 1) * P, :], ost[t][:])

    nc.compile()
    return nc


_CACHE = {}


def _get_nc(cfg):
    key = (cfg.N, cfg.D, cfg.NC, cfg.GT)
    if key not in _CACHE:
        _CACHE[key] = build(cfg)
    return _CACHE[key]


def run(inputs, cfg, **spmd_kwargs):
    """Shard f32 inputs, run the SPMD kernel, gather f32 output."""
    from concourse import bass_utils

    x = np.asarray(inputs["x"], dtype=np.float32)
    x16T = np.ascontiguousarray(x.astype(np.float16).T)          # [D, N]
    w16 = {
        k: np.ascontiguousarray(np.asarray(inputs[k]).astype(np.float16))
        for k in ("w_keys", "w_values", "w_querys")
    }
    NL = cfg.NL
    in_maps = [
        {
            "xT": np.ascontiguousarray(x16T[:, r * NL:(r + 1) * NL]),
            "wk": w16["w_keys"],
            "wv": w16["w_values"],
            "wq": w16["w_querys"],
        }
        for r in range(cfg.NC)
    ]
    nc = _get_nc(cfg)
    res = bass_utils.run_bass_kernel_spmd(
        nc, in_maps, core_ids=list(range(cfg.NC)), **spmd_kwargs
    )
    out = np.concatenate([res.results[r]["out"] for r in range(cfg.NC)], axis=0)
    return out.astype(np.float32, copy=False), res


def kernel(x, w_keys, w_values, w_querys):
    out, _ = run(
        {"x": x, "w_keys": w_keys, "w_values": w_values, "w_querys": w_querys},
        Cfg(),
    )
    return out
